# revision 19
# baseline (speedup 1.0000x reference)
# nn_DirectionalConv on TRN2 (8 NeuronCores), Bass/Tile.
#
#   out[r] = deg_inv[r] * sum_{e: row[e]==r} edge_weight[e] * x[col[e]]
#   x: [100000, 32] f32, edge_index: [2, 1600000] i32 (row=dst, col=src)
#
# Strategy (destination-sharded, batched DMA-gather):
#  * Host sorts destination rows by degree and packs them into blocks of 128
#    rows; blocks are dealt to the 8 cores snake-wise (load balance). Block
#    position g on a core maps its 128 rows onto the 128 SBUF partitions.
#    Row (g, p) owns K_sched[g] edge slots (K_sched = group max degree, so a
#    single NEFF serves all cores).
#  * x is host-packed to bf16 "quad-transposed" records: xq[b][f*4+q] =
#    bf16(x[4b+q][f]) -- 256B per record, so a single int16 index (col//4 <
#    25000) addresses any source row. One InstDMAGatherAnt per span of 64
#    columns gathers 128*ext edges (one 256B record per edge slot) in ONE
#    instruction -- amortizing the ~1us/instr SWDGE cost that made the
#    per-column indirect-DMA baseline issue-bound.
#  * DVE multiplies by a per-(slot, quad) weight (edge_weight * deg_inv,
#    quad-onehot; broadcast over f via a stride-0 AP dim), reduces the quad
#    axis (unit stride) to f32 [P, ext, 32], then segment-reduces each row's
#    K columns -> [128, 32], DMA'd out contiguously.
#  * No scatter, no collectives. The host unpermutes rows at the end.
import numpy as np
import ml_dtypes

P = 128
F = 32
KSPAN = 64
N_CORES = 8
QN = 4          # x rows packed per 256B gather record
GCOLS = 8       # slot columns (128 edges each) per dma_gather instruction
NQUEUES = 4     # SWDGE queues; gathers round-robin to hide ring-drain stalls
IDXW = KSPAN * P // 16  # int16 idx words per partition per full span

LAST_EXEC_TIME_NS = None


def _build_schedule(row, col, w, deg_inv, N):
    global KSPAN
    E = row.shape[0]
    deg = np.bincount(row, minlength=N).astype(np.int64)
    KSPAN = max(64, int(deg.max(initial=0)))  # 64 for the reference input
    B_total = -(-N // (P * N_CORES)) * N_CORES
    N_pad = B_total * P
    deg_pad = np.concatenate([deg, np.zeros(N_pad - N, np.int64)])
    order = np.argsort(deg_pad, kind="stable")
    rank = np.empty(N_pad, np.int64)
    rank[order] = np.arange(N_pad)

    G = B_total // N_CORES
    K_blk = deg_pad[order].reshape(B_total, P).max(axis=1)
    K_sched = K_blk.reshape(G, N_CORES).max(axis=1).astype(np.int64)
    assert K_sched.max(initial=0) <= KSPAN, "block max degree exceeds one span"
    slot_base = np.zeros(G + 1, np.int64)
    acc = 0
    for g in range(G):
        k = int(K_sched[g])
        if k > 0 and (acc % KSPAN) + k > KSPAN:
            acc = -(-acc // KSPAN) * KSPAN
        slot_base[g] = acc
        acc += k
    slot_base[G] = acc
    S_pp = acc
    S_pad = -(-max(S_pp, 1) // KSPAN) * KSPAN

    pr = rank[row]
    blk_e = pr // P
    p_e = (pr % P).astype(np.int64)
    g_e = blk_e // N_CORES
    j_e = blk_e % N_CORES
    core_e = np.where(g_e % 2 == 0, j_e, N_CORES - 1 - j_e)
    if E > 0:
        o = np.argsort(pr, kind="stable")
        pr_s = pr[o]
        first = np.r_[True, pr_s[1:] != pr_s[:-1]]
        run_start = np.maximum.accumulate(np.where(first, np.arange(E), 0))
        k_s = np.arange(E) - run_start
        k_e = np.empty(E, np.int64)
        k_e[o] = k_s
    else:
        k_e = np.zeros(0, np.int64)
    off_e = slot_base[g_e] + k_e

    # per-slot source column (padding slots -> col 0, weight 0)
    col_slots = np.zeros((N_CORES, P, S_pad), np.int32)
    w_slots = np.zeros((N_CORES, P, S_pad), np.float64)
    wdi = w.astype(np.float64) * deg_inv[row].astype(np.float64)
    col_slots[core_e, p_e, off_e] = col
    w_slots[core_e, p_e, off_e] = wdi

    col_used = np.zeros((N_CORES, S_pad), bool)
    col_used[core_e, off_e] = True
    used_any = col_used.any(axis=0)

    n_spans = S_pad // KSPAN
    exts = []
    for s in range(n_spans):
        u = used_any[s * KSPAN:(s + 1) * KSPAN]
        ext = int(np.max(np.nonzero(u)[0]) + 1) if u.any() else 0
        assert u[:ext].all(), "span used columns not a prefix"
        exts.append(ext)

    # --- packed bf16 quad-transposed x: xq[b][f*4+q] = bf16(x[4b+q][f])
    # (built in kernel() since x isn't passed here)

    # --- int16 gather index tables, wrapped-16 + replicated across 8 Q7 cores
    # list position i of span s maps to dst slot (p=i%128, c=s*64+i//128);
    # stored at partition (i%16)+16*g for g in 0..7, word s*IDXW + i//16.
    colq = (col_slots >> 2).astype(np.int16)          # [NC, P, S_pad]
    qsel = (col_slots & 3).astype(np.int64)           # quad lane of each slot
    idx_tab = np.zeros((N_CORES, P, n_spans * IDXW), np.int16)
    for s in range(n_spans):
        ext = exts[s]
        if ext == 0:
            continue
        n = P * ext
        i = np.arange(n)
        vals = colq[:, i % P, s * KSPAN + i // P]     # [NC, n]
        wrd = s * IDXW + i // 16
        lane = i % 16
        for g in range(8):
            idx_tab[:, lane + 16 * g, wrd] = vals

    # --- per-(slot, quad) weights, broadcast over f on-device
    w4 = np.zeros((N_CORES, P, S_pad * 4), np.float32)
    pp, ss = np.meshgrid(np.arange(P), np.arange(S_pad), indexing="ij")
    for c in range(N_CORES):
        w4[c, pp, ss * 4 + qsel[c]] = w_slots[c]
    w4 = w4.astype(ml_dtypes.bfloat16)

    return dict(order=order, K_sched=K_sched, slot_base=slot_base, S_pp=S_pp,
                S_pad=S_pad, G=G, n_spans=n_spans, exts=exts,
                idx_tab=idx_tab, w4=w4, N_pad=N_pad)


def _build_kernel(sched, NQ, gather_bufs=8):
    import concourse.bass as bass
    import concourse.bacc as bacc
    import concourse.tile as tile
    import concourse.mybir as mybir
    from concourse import library_config

    K_sched = sched["K_sched"]
    slot_base = sched["slot_base"]
    S_pad = sched["S_pad"]
    G = sched["G"]
    n_spans = sched["n_spans"]
    exts = sched["exts"]

    nc = bacc.Bacc("TRN2", target_bir_lowering=False, debug=False,
                   num_devices=N_CORES, num_swdge_queues=NQUEUES)

    xq = nc.dram_tensor("xq", [NQ, QN * F], mybir.dt.bfloat16,
                        kind="ExternalInput")
    idxs = nc.dram_tensor("idxs", [P, n_spans * IDXW], mybir.dt.int16,
                          kind="ExternalInput")
    w4 = nc.dram_tensor("w4", [P, S_pad * 4], mybir.dt.bfloat16,
                        kind="ExternalInput")
    out = nc.dram_tensor("out", [G * P, F], mybir.dt.float32,
                         kind="ExternalOutput")

    with tile.TileContext(nc) as tc:
        with (
            tc.tile_pool(name="ip", bufs=1) as ip,
            tc.tile_pool(name="wp", bufs=1) as wp,
            tc.tile_pool(name="gp", bufs=gather_bufs) as gp,
            tc.tile_pool(name="rp", bufs=4) as rp,
        ):
            nc.gpsimd.load_library(library_config.mlp)

            idx_all = ip.tile([P, n_spans * IDXW], mybir.dt.int16)
            nc.sync.dma_start(out=idx_all[:], in_=idxs[:])
            w_all = wp.tile([P, S_pad * 4], mybir.dt.bfloat16)
            nc.sync.dma_start(out=w_all[:], in_=w4[:])

            span_tiles = {}
            qrr = 0
            for s in range(n_spans):
                ext = exts[s]
                if ext == 0:
                    span_tiles[s] = None
                    continue
                g_t = gp.tile([P, KSPAN, QN * F], mybir.dt.bfloat16, tag="g")
                for c0 in range(0, ext, GCOLS):
                    c1 = min(c0 + GCOLS, ext)
                    w0 = s * IDXW + c0 * (P // 16)
                    nc.gpsimd.dma_gather(
                        out_ap=g_t[:, c0:c1, :],
                        in_ap=xq[:],
                        idxs_ap=idx_all[:, w0:w0 + (c1 - c0) * (P // 16)],
                        num_idxs=P * (c1 - c0),
                        num_idxs_reg=P * (c1 - c0),
                        elem_size=QN * F,
                        queue_num=qrr,
                    )
                    qrr = (qrr + 1) % NQUEUES
                    gv = g_t[:, c0:c1, :].rearrange("p c (f q) -> p c f q",
                                                    q=QN)
                    j0 = (s * KSPAN + c0) * 4
                    wap = (w_all[:, j0:j0 + (c1 - c0) * 4]
                           .rearrange("p (c q) -> p c q", q=QN)
                           .unsqueeze(2)
                           .broadcast_to([P, c1 - c0, F, QN]))
                    nc.vector.tensor_tensor(out=gv, in0=gv, in1=wap,
                                            op=mybir.AluOpType.mult)
                span_tiles[s] = g_t

            g = 0
            while g < G:
                k = int(K_sched[g])
                if k == 0:
                    ge = g
                    while ge < G and int(K_sched[ge]) == 0:
                        ge += 1
                    rz = rp.tile([P, F], mybir.dt.float32, tag="r")
                    nc.vector.memset(rz[:], 0.0)
                    for gg in range(g, ge):
                        nc.sync.dma_start(out=out[gg * P:(gg + 1) * P, :],
                                          in_=rz[:])
                    g = ge
                    continue
                s = int(slot_base[g]) // KSPAN
                ge = g + 1
                while (ge < G and int(K_sched[ge]) == k
                       and int(slot_base[ge]) == int(slot_base[ge - 1]) + k
                       and int(slot_base[ge]) // KSPAN == s):
                    ge += 1
                nrun = ge - g
                j0 = int(slot_base[g]) - s * KSPAN
                g_t = span_tiles[s]
                src = g_t[:, j0:j0 + nrun * k, :].rearrange(
                    "p (r k) (f q) -> p r f k q", k=k, q=QN)
                r_t = rp.tile([P, nrun * F], mybir.dt.float32, tag="r")
                nc.vector.tensor_reduce(out=r_t[:], in_=src,
                                        axis=mybir.AxisListType.XY,
                                        op=mybir.AluOpType.add)
                for i, gg in enumerate(range(g, ge)):
                    nc.sync.dma_start(out=out[gg * P:(gg + 1) * P, :],
                                      in_=r_t[:, i * F:(i + 1) * F])
                g = ge

    nc.compile()
    return nc


def _unshard(sched, core_outs, N):
    G = sched["G"]
    order = sched["order"]
    out = np.zeros((N, F), np.float32)
    g_idx = np.arange(G)
    for c in range(N_CORES):
        j = np.where(g_idx % 2 == 0, c, N_CORES - 1 - c)
        blk = g_idx * N_CORES + j
        ranks = (blk[:, None] * P + np.arange(P)).ravel()
        rows = order[ranks]
        mask = rows < N
        out[rows[mask]] = core_outs[c][mask]
    return out


def _pack_xq(x):
    NQ = -(-x.shape[0] // QN)
    xb = np.zeros((NQ * QN, F), ml_dtypes.bfloat16)
    xb[:x.shape[0]] = x.astype(ml_dtypes.bfloat16)
    # xq[b][f*4+q] = xb[4b+q][f]
    return np.ascontiguousarray(
        xb.reshape(NQ, QN, F).transpose(0, 2, 1).reshape(NQ, QN * F))


def kernel(x, edge_index, edge_weight, deg_inv):
    global LAST_EXEC_TIME_NS
    import os
    from concourse.bass_utils import run_bass_kernel_spmd

    x = np.ascontiguousarray(np.asarray(x, dtype=np.float32))
    edge_index = np.asarray(edge_index, dtype=np.int32)
    edge_weight = np.asarray(edge_weight, dtype=np.float32)
    deg_inv = np.asarray(deg_inv, dtype=np.float32)
    N = x.shape[0]

    sched = _build_schedule(edge_index[0], edge_index[1], edge_weight,
                            deg_inv, N)
    xq = _pack_xq(x)
    nc = _build_kernel(sched, xq.shape[0])
    in_maps = [{"xq": xq, "idxs": sched["idx_tab"][c], "w4": sched["w4"][c]}
               for c in range(N_CORES)]

    trace = bool(int(os.environ.get("KERNEL_TRACE", "0")))
    res = run_bass_kernel_spmd(nc, in_maps, core_ids=list(range(N_CORES)),
                               trace=trace)
    if trace:
        LAST_EXEC_TIME_NS = res.exec_time_ns
    return _unshard(sched, [r["out"] for r in res.results], N)
